# revision 1
# baseline (speedup 1.0000x reference)
"""Trainium2 Bass kernel for nn_Attention_interaction (dense_transformer).

Math (per batch b, head h):
    q = l2norm(x);  S = (q @ q^T) / SCALE / attn_gamma;  P = softmax(S, -1)
    o = P @ y;  o2 = o @ W^T + bias;  out = w0*y + w1*o2
with w_i = exp(sum_gamma_i) / (exp(sum_gamma0) + exp(sum_gamma1)).

Sharding: batch dim B=8 across the 8 cores (1 batch x 8 heads per core).
Per core the 8 heads run in 4 pairs (head A's qT operands on SBUF
partitions 0-63, head B's on 64-127, addressed via matmul tile_position).

The kernel is ACT(exp)-bound: softmax needs 8.4M exps per core and the
Scalar engine runs 1 elem/lane/cycle at 1.2 GHz (~55us floor). Everything
is arranged so the exp stream never waits and the PE stays dense (and
therefore HAM-warm):
  - Softmax skips max-subtraction (q rows are unit vectors so logits are
    bounded by 1/(SCALE*gamma)) and exp needs no accumulator: the softmax
    denominators accumulate in the O matmul's 65th output row via a
    ones-column appended to y on the host.
  - Per head the S columns are laid out jc-major (col = jc*4096 + i*512)
    and streamed through [128,1024] PSUM chunks (2-slot round robin, heads
    interleaved). O matmuls are emitted into the chunk loop with a
    one-chunk delay (2 per chunk, evenly) so the static per-engine program
    order is always runnable. PSUM plan (8 banks): S stream 4, per-head
    jc0-bank (O-jc0 then proj blocks 0-3) and jc1-bank (O-jc1 then proj
    blocks 4-7) = 4.
  - q-prep: l2norm via fast-inverse-sqrt + Newton on DVE (no Sqrt/Rsqrt
    tables — only Exp is used, one activation-table load), scale+cast to
    bf16, q^T built by DMA-xbar transposes (bacc's event-semaphore pass
    legalizes the XPOSE single-wait-slot limit).
  - proj = OT^T @ wt_aug with K=65: wt_aug row 64 = w1*bias, so r*w1*bias
    folds into the matmul and the 1/r epilogue scale leaves exactly
    w1*bias; epilogue adds the host-provided w0*y.
  - Denominator rows are moved into [128,1]-per-block layout by a small
    DRAM-bounce scatter DMA (DMA engines are otherwise idle).
"""

import math
import os

import numpy as np
import ml_dtypes

import concourse.bass as bass
import concourse.bacc as bacc
import concourse.tile as tile
from concourse import mybir
from concourse.bass_utils import run_bass_kernel_spmd
from concourse._compat import get_trn_type

B, H, N, D = 8, 8, 1024, 64
SCALE = (512 // 8) ** (-0.5)  # 0.125
EPS = 1e-6
NCORES = 8
NB = N // 128  # 8 row blocks of 128
NW = N * NB  # 8192 flattened S columns per head
CHUNK = 1024  # exp granularity (PSUM columns per ACT instruction)
F32 = mybir.dt.float32
BF16 = mybir.dt.bfloat16
I32 = mybir.dt.int32
AX = mybir.AxisListType
OP = mybir.AluOpType
ACT = mybir.ActivationFunctionType
MAGIC = 0x5F3759DF

LAST_RESULTS = None  # BassKernelResults of the most recent run (for test.py)


def _emit(ctx, tc, sqrt_c2: float):
    """Emit the per-core program. sqrt_c2 = sqrt(1/(SCALE*attn_gamma)) is
    folded into the q row scales so S comes out of the PE pre-scaled."""
    nc = tc.nc
    x_bf = nc.dram_tensor("x_bf", [H, N, D], BF16, kind="ExternalInput")
    ya = nc.dram_tensor("ya", [H, N, D + 1], BF16, kind="ExternalInput")
    yb = nc.dram_tensor("yb", [H, N, D], F32, kind="ExternalInput")
    wt = nc.dram_tensor("wt", [D + 1, D], BF16, kind="ExternalInput")
    out = nc.dram_tensor("out", [H, N, D], F32, kind="ExternalOutput")
    # DRAM bounce buffer for the denominator-row transposes
    rscr = nc.dram_tensor("rscr", [2, 2, N], BF16)

    singles = ctx.enter_context(tc.tile_pool(name="singles", bufs=1))
    io = ctx.enter_context(tc.tile_pool(name="io", bufs=2))
    st = ctx.enter_context(tc.tile_pool(name="st", bufs=2))
    work = ctx.enter_context(tc.tile_pool(name="work", bufs=2))
    epool = ctx.enter_context(tc.tile_pool(name="epool", bufs=2))
    qpool = ctx.enter_context(tc.tile_pool(name="qpool", bufs=1))
    # PSUM: 8 banks = S stream 2x[128,1024] (4) + per-head jc0/jc1 banks (4)
    ps_s = ctx.enter_context(tc.tile_pool(name="ps_s", bufs=2, space="PSUM"))
    ps_o = ctx.enter_context(tc.tile_pool(name="ps_o", bufs=1, space="PSUM"))

    # proj weight (rows 0-63 = w1*W^T, row 64 = w1*bias)
    wt_sb = singles.tile([D + 1, D], BF16)
    nc.sync.dma_start(out=wt_sb, in_=wt[:, :])

    qT = [None] * (H // 2)

    def prep(p):
        """Loads + l2norm + q scale/cast + DMA-transpose into qT[p].
        Processed in two block groups so pair 0's first S chunk (which only
        needs qT blocks 0-3) is ready as early as possible."""
        hA, hB = 2 * p, 2 * p + 1
        xA = io.tile([128, NB, D], BF16, tag="xA")
        xB = io.tile([128, NB, D], BF16, tag="xB")
        nc.sync.dma_start(out=xA, in_=x_bf[hA].rearrange("(b p) d -> p b d", p=128))
        nc.sync.dma_start(out=xB, in_=x_bf[hB].rearrange("(b p) d -> p b d", p=128))

        q = qpool.tile([128, N], BF16, tag=f"qT{p}", name=f"qT{p}")
        hb = NB // 2
        for g in range(2):
            b0 = g * hb
            # row norms for blocks b0..b0+3 of both heads:
            # ss[:, 0:4] = head A, ss[:, 4:8] = head B
            ss = st.tile([128, 2 * hb], F32, tag="ss")
            sqA = st.tile([128, hb, D], F32, tag="sqA")
            sqB = st.tile([128, hb, D], F32, tag="sqB")
            nc.vector.tensor_mul(sqA, xA[:, b0 : b0 + hb, :], xA[:, b0 : b0 + hb, :])
            nc.vector.reduce_sum(ss[:, 0:hb], sqA, axis=AX.X)
            nc.vector.tensor_mul(sqB, xB[:, b0 : b0 + hb, :], xB[:, b0 : b0 + hb, :])
            nc.vector.reduce_sum(ss[:, hb : 2 * hb], sqB, axis=AX.X)

            # rs = sqrt_c2 / sqrt(ss + eps): fast inverse sqrt + 3 Newton
            half = st.tile([128, 2 * hb], F32, tag="half")
            nc.vector.tensor_scalar(
                out=half, in0=ss, scalar1=0.5, scalar2=0.5 * EPS,
                op0=OP.mult, op1=OP.add,
            )
            yv = st.tile([128, 2 * hb], F32, tag="yv")
            yi = yv.bitcast(I32)
            nc.vector.tensor_scalar(
                out=yi, in0=ss.bitcast(I32), scalar1=1, scalar2=None,
                op0=OP.logical_shift_right,
            )
            nc.vector.tensor_scalar(
                out=yi, in0=yi, scalar1=MAGIC, scalar2=-1,
                op0=OP.subtract, op1=OP.mult,
            )
            t1 = st.tile([128, 2 * hb], F32, tag="t1")
            for it in range(3):
                last = it == 2
                nc.vector.tensor_mul(t1, yv, yv)
                nc.vector.tensor_mul(t1, t1, half)
                nc.vector.tensor_scalar(
                    out=t1, in0=t1, scalar1=1.5,
                    scalar2=(-sqrt_c2 if last else -1.0),
                    op0=OP.subtract, op1=OP.mult,
                )
                nc.vector.tensor_mul(yv, yv, t1)

            # q blocks (bf16), interleaved [A-dims | B-dims] per 128-col
            # group, then DMA-xbar transpose into qT
            qAB = work.tile([128, hb, 128], BF16, tag="qAB")
            for b in range(hb):
                nc.vector.tensor_scalar_mul(
                    out=qAB[:, b, 0:D], in0=xA[:, b0 + b, :],
                    scalar1=yv[:, b : b + 1],
                )
                nc.vector.tensor_scalar_mul(
                    out=qAB[:, b, D:128], in0=xB[:, b0 + b, :],
                    scalar1=yv[:, hb + b : hb + b + 1],
                )
            for b in range(hb):
                nc.sync.dma_start(
                    out=q[:, (b0 + b) * 128 : (b0 + b + 1) * 128],
                    in_=qAB[:, b],
                    transpose=True,
                )
        qT[p] = q

    prep(0)
    prep(1)

    for p in range(H // 2):
        hA, hB = 2 * p, 2 * p + 1
        q = qT[p]

        yA = io.tile([128, NB, D + 1], BF16, tag="yA")
        yB = io.tile([128, NB, D + 1], BF16, tag="yB")
        ybA = io.tile([128, NB, D], F32, tag="ybA")
        ybB = io.tile([128, NB, D], F32, tag="ybB")
        nc.sync.dma_start(out=yA, in_=ya[hA].rearrange("(b p) d -> p b d", p=128))
        nc.sync.dma_start(out=yB, in_=ya[hB].rearrange("(b p) d -> p b d", p=128))
        nc.sync.dma_start(out=ybA, in_=yb[hA].rearrange("(b p) d -> p b d", p=128))
        nc.sync.dma_start(out=ybB, in_=yb[hB].rearrange("(b p) d -> p b d", p=128))

        EA = epool.tile([128, NW], BF16, tag="EA")
        EB = epool.tile([128, NW], BF16, tag="EB")
        OTA = work.tile([D + 1, N], BF16, tag="OTA")
        OTB = work.tile([D + 1, N], BF16, tag="OTB")
        heads = (
            (0, EA, yA, OTA),
            (64, EB, yB, OTB),
        )
        okptr = [0, 0]  # per head: next O matmul (jc-major index jc*8+i)
        otile = [None, None]

        def emit_o(hidx, limit):
            """Emit O matmuls whose E input (cols < limit) is ready. The
            65th output row accumulates the softmax denominators."""
            base, E, ytile, OT = heads[hidx]
            hc = "AB"[hidx]
            while okptr[hidx] < 16:
                k = okptr[hidx]
                jc, i = k // NB, k % NB
                if jc * 4096 + (i + 1) * 512 > limit:
                    return
                if i == 0:
                    otile[hidx] = ps_o.tile(
                        [128, 512], F32, tag=f"o{jc}{hc}", name=f"ot{jc}{hc}"
                    )
                nc.tensor.matmul(
                    otile[hidx][0 : D + 1, :],
                    lhsT=ytile[:, i, :],
                    rhs=E[:, jc * 4096 + i * 512 : jc * 4096 + (i + 1) * 512],
                    start=(i == 0), stop=(i == NB - 1), tile_position=(0, 0),
                )
                if i == NB - 1:
                    nc.vector.tensor_copy(
                        OT[:, jc * 512 : (jc + 1) * 512],
                        otile[hidx][0 : D + 1, :],
                    )
                okptr[hidx] += 1

        def emit_proj(hidx, jc):
            """proj for output blocks jc*4..jc*4+3 (needs OT cols of that jc
            half); lands in the jc bank this head just freed."""
            base, E, ytile, OT = heads[hidx]
            hc = "AB"[hidx]
            pj = ps_o.tile([128, 512], F32, tag=f"o{jc}{hc}", name=f"pj{jc}{hc}")
            for b in range(jc * 4, jc * 4 + 4):
                nc.tensor.matmul(
                    pj[:, (b - jc * 4) * 128 : (b - jc * 4) * 128 + D],
                    lhsT=OT[:, b * 128 : (b + 1) * 128],
                    rhs=wt_sb,
                    start=True, stop=True, tile_position=(0, 0),
                )
            return pj

        pjs = [[None, None], [None, None]]  # [hidx][jc]
        # ---- S/exp chunk stream with O interleaved (one-chunk delay) ----
        for c in range(NW // CHUNK):
            jc, ip = c // 4, (c % 4) * 2
            for hidx, (base, E, ytile, OT) in enumerate(heads):
                ps = ps_s.tile([128, CHUNK], F32, tag="psS", name="psS")
                for i in (ip, ip + 1):
                    nc.tensor.matmul(
                        ps[:, (i - ip) * 512 : (i - ip + 1) * 512],
                        lhsT=q[base : base + 64, i * 128 : (i + 1) * 128],
                        rhs=q[base : base + 64, jc * 512 : (jc + 1) * 512],
                        start=True, stop=True, tile_position=(base, 0),
                    )
                nc.scalar.activation(
                    out=E[:, c * CHUNK : (c + 1) * CHUNK], in_=ps, func=ACT.Exp
                )
                emit_o(hidx, c * CHUNK)
                if c == 4:
                    # jc0 accumulation evacuated at c==4's emit_o; its bank
                    # is free — run the first proj half here.
                    pjs[hidx][0] = emit_proj(hidx, 0)

        # ---- pair tail: O flush, denominators, proj half 2, epilogue ----
        rT = st.tile([128, 2, NB], BF16, tag="rT")
        rinv = st.tile([128, 2 * NB], F32, tag="rinv")
        for hidx, (base, E, ytile, OT) in enumerate(heads):
            emit_o(hidx, NW)
            nc.sync.dma_start(out=rscr[p % 2, hidx], in_=OT[D : D + 1, :])
            nc.sync.dma_start(
                out=rT[:, hidx, :],
                in_=rscr[p % 2, hidx].rearrange("(b p) -> p b", p=128),
            )
            pjs[hidx][1] = emit_proj(hidx, 1)
        nc.vector.reciprocal(rinv, rT.rearrange("p a b -> p (a b)"))

        for hidx, o2t, ybt, fint, ho in (
            (0, "o2A", "ybA", "finA", hA),
            (1, "o2B", "ybB", "finB", hB),
        ):
            o2 = work.tile([128, NB, D], F32, tag=o2t, name=o2t)
            for b in range(NB):
                nc.vector.tensor_scalar_mul(
                    out=o2[:, b, :],
                    in0=pjs[hidx][b // 4][:, (b % 4) * 128 : (b % 4) * 128 + D],
                    scalar1=rinv[:, hidx * NB + b : hidx * NB + b + 1],
                )
            fin = work.tile([128, NB, D], F32, tag=fint, name=fint)
            nc.vector.tensor_add(fin, o2, ybA if hidx == 0 else ybB)
            nc.sync.dma_start(
                out=out[ho].rearrange("(b p) d -> p b d", p=128), in_=fin
            )

        if p + 2 < H // 2:
            prep(p + 2)


def build_program(sqrt_c2: float) -> bass.Bass:
    from contextlib import ExitStack

    nc = bacc.Bacc(get_trn_type() or "TRN2", target_bir_lowering=False)
    with tile.TileContext(nc) as tc:
        with ExitStack() as ctx:
            _emit(ctx, tc, sqrt_c2)
    # bacc passes legalize sync waits (≤1 wait per instruction on TRN2) and
    # insert the activation-table loads.
    nc.compile()
    return nc


def kernel(x, y, proj_w, proj_b, attn_gamma, sum_gamma0, sum_gamma1):
    global LAST_RESULTS
    x = np.asarray(x, dtype=np.float32)
    y = np.asarray(y, dtype=np.float32)
    proj_w = np.asarray(proj_w, dtype=np.float32)
    proj_b = np.asarray(proj_b, dtype=np.float32)
    g0 = math.exp(float(np.asarray(sum_gamma0)))
    g1 = math.exp(float(np.asarray(sum_gamma1)))
    w0 = g0 / (g0 + g1)
    w1 = g1 / (g0 + g1)
    c2 = 1.0 / (SCALE * float(np.asarray(attn_gamma)))

    nc = build_program(math.sqrt(c2))

    x_bf = x.astype(ml_dtypes.bfloat16)
    # y with a ones column appended: the O matmul's 65th output row then
    # accumulates the softmax denominators.
    ya = np.concatenate(
        [y, np.ones(y.shape[:-1] + (1,), np.float32)], axis=-1
    ).astype(ml_dtypes.bfloat16)
    yb = (w0 * y).astype(np.float32)
    # wt rows 0-63 = w1*W^T; row 64 = w1*bias (multiplies the r row, so the
    # 1/r epilogue scale leaves exactly w1*bias).
    wt = np.concatenate([proj_w.T * w1, w1 * proj_b[None, :]], axis=0).astype(
        ml_dtypes.bfloat16
    )

    in_maps = [
        {"x_bf": x_bf[c], "ya": ya[c], "yb": yb[c], "wt": wt}
        for c in range(NCORES)
    ]
    res = run_bass_kernel_spmd(nc, in_maps, list(range(NCORES)))
    LAST_RESULTS = res
    return np.stack([res.results[c]["out"] for c in range(NCORES)], axis=0)



# revision 6
# speedup vs baseline: 1.0400x; 1.0400x over previous
"""Trainium2 Bass kernel for nn_Attention_interaction (dense_transformer).

Math (per batch b, head h):
    q = l2norm(x);  S = (q @ q^T) / SCALE / attn_gamma;  P = softmax(S, -1)
    o = P @ y;  o2 = o @ W^T + bias;  out = w0*y + w1*o2
with w_i = exp(sum_gamma_i) / (exp(sum_gamma0) + exp(sum_gamma1)).

Sharding: batch dim B=8 across the 8 cores (1 batch x 8 heads per core).

v2 design (vs the v1 baseline at ~168-191us):
  - S matmuls run in fp8e4 DoubleRow mode (2 fp8 K-values packed per 16-bit
    element): q is scaled+cast to fp8 token-major, then a 2-byte DMA-xbar
    transpose of the packed pairs directly yields the [32, 2, t] DoubleRow
    operand layout. 4 heads' qT coexist at PE quadrant rows 0/32/64/96.
  - exp is split between ACT (native Exp) and DVE (Schraudolph on the bf16
    bit pattern: bits16 = round(s*128/ln2 + (127*128 - sigma)), bitcast to
    bf16; sigma tuned for min max-rel-err ~3.3%, far inside the 2e-2 gate).
  - Softmax denominators accumulate in the O matmul's 65th row via a
    ones-column appended to y on the host (as v1); r goes through a DRAM
    bounce to become per-partition [128, 8] and a single reciprocal.
  - O^T = ya^T E uses E symmetry (as v1). Per head O accumulates into ONE
    [128,1024] PSUM tile; after each jc-half is evacuated (ACT for jc0, DVE
    for jc1), the proj matmuls write pj into the freed half of the same
    tile - PSUM stays at 8 banks with a 2-deep S-chunk stream.
  - Epilogue: one DVE tensor_mul per head (pj x rinv broadcast along D via
    a stride-0 AP), then fin = o2 + w0*y on GpSimd (SBUF-only engine), then
    store. q-norm squares for heads 4-7 and their q-scales also run on
    GpSimd; Newton inverse-sqrt + reduces stay on DVE in packed shapes.
"""

import math

import numpy as np
import ml_dtypes

import concourse.bass as bass
import concourse.bacc as bacc
import concourse.tile as tile
from concourse import mybir
from concourse.bass_utils import run_bass_kernel_spmd
from concourse._compat import get_trn_type

B, H, N, D = 8, 8, 1024, 64
SCALE = (512 // 8) ** (-0.5)  # 0.125
EPS = 1e-6
NCORES = 8
NB = N // 128  # 8 row blocks of 128
NW = N * NB  # 8192 flattened S columns per head
CHUNK = 1024
F32 = mybir.dt.float32
BF16 = mybir.dt.bfloat16
FP8 = mybir.dt.float8e4
I16 = mybir.dt.int16
I32 = mybir.dt.int32
AX = mybir.AxisListType
OP = mybir.AluOpType
ACT = mybir.ActivationFunctionType
PM = mybir.MatmulPerfMode
MAGIC = 0x5F3759DF

# Schraudolph exp on bf16 bit patterns: bits = round(s*A_SCH + B_SCH)
A_SCH = 128.0 / math.log(2.0)
SIGMA = 5.5  # tuned for min max-rel-err under round-to-nearest
B_SCH = 127.0 * 128.0 - SIGMA

# exp engine schedule per pair: chunk-slots (0..15) run on DVE; rest on ACT.
# Pair 0 gives DVE less exp work (it also does group-0 q-prep during warmup).
DVE_SLOTS = [
    {3, 6, 9, 11, 13, 15},
    {1, 3, 5, 8, 10, 12, 14},
    {1, 3, 5, 8, 10, 12, 14},
    {1, 3, 5, 8, 10, 12, 14},
]

LAST_RESULTS = None  # BassKernelResults of the most recent run (for test.py)


def _emit(ctx, tc, sqrt_c2: float):
    nc = tc.nc
    xq = nc.dram_tensor("xq", [H, N, D], BF16, kind="ExternalInput")
    ya = nc.dram_tensor("ya", [H, N, D + 1], BF16, kind="ExternalInput")
    yb = nc.dram_tensor("yb", [H, N, D], F32, kind="ExternalInput")
    wt = nc.dram_tensor("wt", [D + 1, D], BF16, kind="ExternalInput")
    out = nc.dram_tensor("out", [H, N, D], F32, kind="ExternalOutput")
    rscr = nc.dram_tensor("rscr", [4, 2, N], BF16)  # denominator bounce

    singles = ctx.enter_context(tc.tile_pool(name="singles", bufs=1))
    io = ctx.enter_context(tc.tile_pool(name="io", bufs=2))
    st = ctx.enter_context(tc.tile_pool(name="st", bufs=2))
    qpool = ctx.enter_context(tc.tile_pool(name="qpool", bufs=1))
    epool = ctx.enter_context(tc.tile_pool(name="epool", bufs=2))
    wpool = ctx.enter_context(tc.tile_pool(name="wpool", bufs=2))
    ps_s = ctx.enter_context(tc.tile_pool(name="ps_s", bufs=2, space="PSUM"))
    ps_o = ctx.enter_context(tc.tile_pool(name="ps_o", bufs=1, space="PSUM"))

    qT4 = [None, None]  # per group: [128, NB, 128] bf16 (fp8-pair packed)

    def qprep(g):
        """q-prep for heads 4g..4g+3: load, norms, scale+cast fp8, DMA-xbar
        transpose into DoubleRow layout. Squares+scales on DVE for group 0
        (warmup, DVE idle) and on GpSimd for group 1 (steady state)."""
        eng = nc.vector if g == 0 else nc.gpsimd
        x4 = io.tile([128, 4, NB, D], BF16, tag=f"x4_{g}")
        nc.sync.dma_start(
            out=x4,
            in_=xq[4 * g : 4 * g + 4].rearrange("h (b p) d -> p h b d", p=128),
        )
        ss = st.tile([128, 4, NB], F32, tag=f"ss{g}")
        for hi in range(4):
            sq = st.tile([128, NB, D], BF16, tag=f"sq{g}")
            eng.tensor_mul(sq, x4[:, hi], x4[:, hi])
            nc.vector.tensor_reduce(ss[:, hi], sq, axis=AX.X, op=OP.add)

        # rs = sqrt_c2 / sqrt(ss + eps): fast inverse sqrt + 2 Newton (DVE)
        ssf = ss.rearrange("p h b -> p (h b)")
        half = st.tile([128, 32], F32, tag=f"half{g}")
        nc.vector.tensor_scalar(
            out=half, in0=ssf, scalar1=0.5, scalar2=0.5 * EPS,
            op0=OP.mult, op1=OP.add,
        )
        rs = st.tile([128, 32], F32, tag=f"rs{g}")
        yi = rs.bitcast(I32)
        nc.vector.tensor_scalar(
            out=yi, in0=ssf.bitcast(I32), scalar1=1, scalar2=None,
            op0=OP.logical_shift_right,
        )
        nc.vector.tensor_scalar(
            out=yi, in0=yi, scalar1=MAGIC, scalar2=-1,
            op0=OP.subtract, op1=OP.mult,
        )
        t1 = st.tile([128, 32], F32, tag=f"t1{g}")
        for it in range(2):
            last = it == 1
            nc.vector.tensor_mul(t1, rs, rs)
            nc.vector.tensor_mul(t1, t1, half)
            nc.vector.tensor_scalar(
                out=t1, in0=t1, scalar1=1.5,
                scalar2=(-sqrt_c2 if last else -1.0),
                op0=OP.subtract, op1=OP.mult,
            )
            nc.vector.tensor_mul(rs, rs, t1)

        # scale+cast to fp8 into 4-head packed blocks, then transpose per
        # block: [128 tok, 128] uint16-pairs -> [128 (4h x 32 dpair), 128 tok].
        # The 2-byte xbar transpose leaves the d-pair bytes token-interleaved;
        # walrus DoubleRow needs each k-tile's token run contiguous, so a
        # GpSimd deinterleave pass converts to the [p, ktile, token] block
        # layout (k = (p, ktile) covers d = 2p+ktile on both operands).
        q4 = qpool.tile([128, NB, 128], BF16, tag=f"q4_{g}")
        qTi = qpool.tile([128, NB, 128], BF16, tag=f"qTi_{g}", name=f"qTi_{g}")
        qT = qpool.tile([128, N], BF16, tag=f"qT4_{g}", name=f"qT4_{g}")
        q4f = q4.bitcast(FP8)  # [128, NB, 256]
        qTif = qTi.bitcast(FP8)  # [128, NB, 256]
        qTf = qT.bitcast(FP8).rearrange("p (two t) -> p two t", two=2)
        for b in range(NB):
            for hi in range(4):
                eng.tensor_scalar_mul(
                    out=q4f[:, b, hi * 64 : (hi + 1) * 64],
                    in0=x4[:, hi, b, :],
                    scalar1=rs[:, hi * NB + b : hi * NB + b + 1],
                )
            nc.sync.dma_start(out=qTi[:, b, :], in_=q4[:, b, :], transpose=True)
            if b % 4 == 3:
                b0 = b - 3
                nc.gpsimd.tensor_copy(
                    qTf[:, :, b0 * 128 : (b0 + 4) * 128],
                    qTif[:, b0 : b0 + 4, :].rearrange(
                        "p b (t two) -> p two (b t)", two=2
                    ),
                )
        qT4[g] = qT

    def q_lhsT(g, hi, i):
        f = qT4[g].bitcast(FP8).rearrange("p (two t) -> p two t", two=2)
        return f[hi * 32 : (hi + 1) * 32, :, i * 128 : (i + 1) * 128]

    def q_rhs(g, hi, jc):
        f = qT4[g].bitcast(FP8).rearrange("p (two t) -> p two t", two=2)
        return f[hi * 32 : (hi + 1) * 32, :, jc * 512 : (jc + 1) * 512]

    # proj weight (rows 0-63 = w1*W^T, row 64 = w1*bias)
    wt_sb = singles.tile([D + 1, D], BF16)

    qprep(0)
    nc.sync.dma_start(out=wt_sb, in_=wt[:, :])

    for p in range(4):
        g = p // 2
        heads = (2 * p, 2 * p + 1)  # global head ids
        his = (heads[0] % 4, heads[1] % 4)  # index within group

        ya_t = [None, None]
        yb_t = [None, None]
        for hidx in range(2):
            ya_t[hidx] = io.tile(
                [128, NB, D + 1], BF16, tag=f"ya{hidx}", name=f"ya{hidx}"
            )
            nc.sync.dma_start(
                out=ya_t[hidx],
                in_=ya[heads[hidx]].rearrange("(b p) d -> p b d", p=128),
            )
            yb_t[hidx] = io.tile(
                [128, NB, D], F32, tag=f"yb{hidx}", name=f"yb{hidx}"
            )
            nc.sync.dma_start(
                out=yb_t[hidx],
                in_=yb[heads[hidx]].rearrange("(b p) d -> p b d", p=128),
            )

        E = [
            epool.tile([128, NW], BF16, tag="EA", name=f"EA{p}"),
            epool.tile([128, NW], BF16, tag="EB", name=f"EB{p}"),
        ]
        po = [
            ps_o.tile([128, 1024], F32, tag="oA", name=f"oA{p}"),
            ps_o.tile([128, 1024], F32, tag="oB", name=f"oB{p}"),
        ]
        OT = [
            wpool.tile([D + 1, N], BF16, tag="OTA", name=f"OTA{p}"),
            wpool.tile([D + 1, N], BF16, tag="OTB", name=f"OTB{p}"),
        ]
        okptr = [0, 0]

        def emit_o(hidx, limit):
            """O^T matmuls whose E input (cols < limit) is ready. Row 64
            accumulates the softmax denominators (ones column of ya)."""
            while okptr[hidx] < 16:
                k = okptr[hidx]
                jc, i = k // NB, k % NB
                if jc * 4096 + (i + 1) * 512 > limit:
                    return
                nc.tensor.matmul(
                    po[hidx][0 : D + 1, jc * 512 : (jc + 1) * 512],
                    lhsT=ya_t[hidx][:, i, :],
                    rhs=E[hidx][:, jc * 4096 + i * 512 : jc * 4096 + (i + 1) * 512],
                    start=(i == 0), stop=(i == NB - 1), tile_position=(0, 0),
                )
                okptr[hidx] += 1

        def evac(hidx, jc, use_act):
            src = po[hidx][0 : D + 1, jc * 512 : (jc + 1) * 512]
            dst = OT[hidx][:, jc * 512 : (jc + 1) * 512]
            if use_act:
                nc.scalar.copy(out=dst, in_=src)
            else:
                nc.vector.tensor_copy(dst, src)

        def emit_proj(hidx, jc):
            """proj for token blocks jc*4..jc*4+3 into the freed jc-half of
            this head's O PSUM tile."""
            for bb in range(4):
                b = jc * 4 + bb
                nc.tensor.matmul(
                    po[hidx][:, jc * 512 + bb * 64 : jc * 512 + (bb + 1) * 64],
                    lhsT=OT[hidx][:, b * 128 : (b + 1) * 128],
                    rhs=wt_sb,
                    start=True, stop=True, tile_position=(0, 0),
                )

        # ---- S/exp chunk stream with O interleaved (one-chunk delay) ----
        dve_slots = DVE_SLOTS[p]
        for c in range(NB):
            jc, ip = c // 4, (c % 4) * 2
            for hidx in range(2):
                slot = c * 2 + hidx
                ps = ps_s.tile([128, CHUNK], F32, tag="psS", name="psS")
                for k in range(2):
                    nc.tensor.matmul(
                        ps[:, k * 512 : (k + 1) * 512],
                        lhsT=q_lhsT(g, his[hidx], ip + k),
                        rhs=q_rhs(g, his[hidx], jc),
                        start=True, stop=True,
                        perf_mode=PM.DoubleRow,
                        tile_position=(his[hidx] * 32, 0),
                    )
                if slot in dve_slots:
                    nc.vector.tensor_scalar(
                        out=E[hidx].bitcast(I16)[:, c * CHUNK : (c + 1) * CHUNK],
                        in0=ps, scalar1=A_SCH, scalar2=B_SCH,
                        op0=OP.mult, op1=OP.add,
                    )
                else:
                    nc.scalar.activation(
                        out=E[hidx][:, c * CHUNK : (c + 1) * CHUNK],
                        in_=ps, func=ACT.Exp,
                    )
                emit_o(hidx, c * CHUNK)
                if c == 5:
                    evac(hidx, 0, use_act=(hidx == 0))
                if c == 6:
                    emit_proj(hidx, 0)
        if p == 1:
            qprep(1)

        # ---- pair tail: O flush, denominators, second half, epilogue ----
        rT = st.tile([128, 2, NB], BF16, tag="rT")
        for hidx in range(2):
            emit_o(hidx, NW)
            evac(hidx, 1, use_act=(hidx == 0))
            nc.sync.dma_start(out=rscr[p, hidx], in_=OT[hidx][D : D + 1, :])
            nc.sync.dma_start(
                out=rT[:, hidx, :],
                in_=rscr[p, hidx].rearrange("(b p) -> p b", p=128),
            )
            emit_proj(hidx, 1)

        rinv = st.tile([128, 16, 1], F32, tag="rinv")
        nc.vector.reciprocal(
            rinv.rearrange("p a one -> p (a one)"),
            rT.rearrange("p a b -> p (a b)"),
        )

        for hidx in range(2):
            # o2 = pj * (1/r): pj sits at po cols jc*512 + bb*64 (+64)
            pj = (
                po[hidx]
                .rearrange("p (jc x) -> p jc x", jc=2)[:, :, 0:256]
                .rearrange("p jc (bb d) -> p jc bb d", bb=4)
            )
            rb = (
                rinv[:, hidx * NB : (hidx + 1) * NB, :]
                .rearrange("p (jc bb) one -> p jc bb one", jc=2)
                .broadcast_to([128, 2, 4, D])
            )
            o2 = wpool.tile([128, NB, D], F32, tag=f"o2{hidx}", name=f"o2{hidx}")
            nc.vector.tensor_mul(
                o2.rearrange("p (jc bb) d -> p jc bb d", jc=2), pj, rb
            )
            fin = wpool.tile([128, NB, D], F32, tag=f"fin{hidx}", name=f"fin{hidx}")
            nc.gpsimd.tensor_add(fin, o2, yb_t[hidx])
            nc.sync.dma_start(
                out=out[heads[hidx]].rearrange("(b p) d -> p b d", p=128), in_=fin
            )


def build_program(sqrt_c2: float) -> bass.Bass:
    from contextlib import ExitStack

    nc = bacc.Bacc(get_trn_type() or "TRN2", target_bir_lowering=False)
    with tile.TileContext(nc) as tc:
        with ExitStack() as ctx:
            _emit(ctx, tc, sqrt_c2)
    nc.compile()
    return nc


def make_inputs(x, y, proj_w, proj_b, attn_gamma, sum_gamma0, sum_gamma1):
    x = np.asarray(x, dtype=np.float32)
    y = np.asarray(y, dtype=np.float32)
    proj_w = np.asarray(proj_w, dtype=np.float32)
    proj_b = np.asarray(proj_b, dtype=np.float32)
    g0 = math.exp(float(np.asarray(sum_gamma0)))
    g1 = math.exp(float(np.asarray(sum_gamma1)))
    w0 = g0 / (g0 + g1)
    w1 = g1 / (g0 + g1)
    c2 = 1.0 / (SCALE * float(np.asarray(attn_gamma)))

    xq = x.astype(ml_dtypes.bfloat16)
    yac = np.concatenate(
        [y, np.ones(y.shape[:-1] + (1,), np.float32)], axis=-1
    ).astype(ml_dtypes.bfloat16)
    ybv = (w0 * y).astype(np.float32)
    wtv = np.concatenate([proj_w.T * w1, w1 * proj_b[None, :]], axis=0).astype(
        ml_dtypes.bfloat16
    )
    in_maps = [
        {"xq": xq[c], "ya": yac[c], "yb": ybv[c], "wt": wtv}
        for c in range(NCORES)
    ]
    return in_maps, math.sqrt(c2)


def kernel(x, y, proj_w, proj_b, attn_gamma, sum_gamma0, sum_gamma1):
    global LAST_RESULTS
    in_maps, sqrt_c2 = make_inputs(
        x, y, proj_w, proj_b, attn_gamma, sum_gamma0, sum_gamma1
    )
    nc = build_program(sqrt_c2)
    res = run_bass_kernel_spmd(nc, in_maps, list(range(NCORES)))
    LAST_RESULTS = res
    return np.stack([res.results[c]["out"] for c in range(NCORES)], axis=0)


# revision 7
# speedup vs baseline: 1.0855x; 1.0438x over previous
"""Trainium2 Bass kernel for nn_Attention_interaction (dense_transformer).

Math (per batch b, head h):
    q = l2norm(x);  S = (q @ q^T) / SCALE / attn_gamma;  P = softmax(S, -1)
    o = P @ y;  o2 = o @ W^T + bias;  out = w0*y + w1*o2
with w_i = exp(sum_gamma_i) / (exp(sum_gamma0) + exp(sum_gamma1)).

Sharding: batch dim B=8 across the 8 cores (1 batch x 8 heads per core).

v2 design notes:
  - S matmuls run in fp8e4 DoubleRow mode (contraction k=(partition,ktile),
    d = 2p+ktile). q is scaled+cast to fp8 token-major, 2-byte DMA-xbar
    transposes give a pair-interleaved [32,*] layout, and a GpSimd
    deinterleave pass produces the walrus-required [p, ktile, token-run]
    block layout. 4 heads' operands live at PE quadrant rows 0/32/64/96.
  - exp splits between ACT (native Exp) and DVE (Schraudolph on bf16 bit
    patterns: bits16 = round(s*128/ln2 + (127*128 - sigma)), ~3.3% max rel
    err, way inside the 2e-2 gate), balanced by a greedy time counter.
  - Heads run SEQUENTIALLY (not in pairs): PSUM = 3 S-chunk slots (6 banks)
    + one [128,1024] O tile (2 banks). More slots give the PE runway to
    stay continuously busy (p-state ramp to 2.4 GHz needs 3us of
    uninterrupted execution).
  - Softmax denominators ride in the O matmul's 65th row (ones column in
    ya); proj writes pj into the freed jc-half of the same O PSUM tile.
  - Epilogue: o2 = pj * rinv via one DVE tensor_mul with a stride-0
    broadcast AP; fin = o2 + w0*y on GpSimd; store.
  - q-norm squares/scales: group 0 on DVE (warmup, DVE idle), group 1 on
    GpSimd (steady state); Newton inverse-sqrt (1 iter) on DVE.
"""

import math

import numpy as np
import ml_dtypes

import concourse.bass as bass
import concourse.bacc as bacc
import concourse.tile as tile
from concourse import mybir
from concourse.bass_utils import run_bass_kernel_spmd
from concourse._compat import get_trn_type

B, H, N, D = 8, 8, 1024, 64
SCALE = (512 // 8) ** (-0.5)  # 0.125
EPS = 1e-6
NCORES = 8
NB = N // 128  # 8 row blocks of 128
NW = N * NB  # 8192 flattened S columns per head
CHUNK = 1024
F32 = mybir.dt.float32
BF16 = mybir.dt.bfloat16
FP8 = mybir.dt.float8e4
I16 = mybir.dt.int16
I32 = mybir.dt.int32
AX = mybir.AxisListType
OP = mybir.AluOpType
ACT = mybir.ActivationFunctionType
PM = mybir.MatmulPerfMode
MAGIC = 0x5F3759DF

# Schraudolph exp on bf16 bit patterns: bits = round(s*A_SCH + B_SCH)
A_SCH = 128.0 / math.log(2.0)
SIGMA = 5.5
B_SCH = 127.0 * 128.0 - SIGMA

# greedy exp-engine balancing: estimated op costs in us
ACT_CHUNK = 1.223
DVE_CHUNK = 1.317
ACT_EVAC = 0.80
DVE_EVAC = 0.69
DVE_O2 = 0.78
DVE_RECIP = 0.30

LAST_RESULTS = None  # BassKernelResults of the most recent run (for test.py)


def _emit(ctx, tc, sqrt_c2: float):
    nc = tc.nc
    xq = nc.dram_tensor("xq", [H, N, D], BF16, kind="ExternalInput")
    ya = nc.dram_tensor("ya", [H, N, D + 1], BF16, kind="ExternalInput")
    yb = nc.dram_tensor("yb", [H, N, D], F32, kind="ExternalInput")
    wt = nc.dram_tensor("wt", [D + 1, D], BF16, kind="ExternalInput")
    out = nc.dram_tensor("out", [H, N, D], F32, kind="ExternalOutput")
    rscr = nc.dram_tensor("rscr", [H, N], BF16)  # denominator bounce

    singles = ctx.enter_context(tc.tile_pool(name="singles", bufs=1))
    io = ctx.enter_context(tc.tile_pool(name="io", bufs=2))
    st = ctx.enter_context(tc.tile_pool(name="st", bufs=2))
    qpool = ctx.enter_context(tc.tile_pool(name="qpool", bufs=1))
    epool = ctx.enter_context(tc.tile_pool(name="epool", bufs=2))
    wpool = ctx.enter_context(tc.tile_pool(name="wpool", bufs=2))
    ps_s = ctx.enter_context(tc.tile_pool(name="ps_s", bufs=3, space="PSUM"))
    ps_o = ctx.enter_context(tc.tile_pool(name="ps_o", bufs=1, space="PSUM"))

    qT4 = [None, None]  # per group: [128, 2048] fp8 block layout (as bf16 tile)
    eng_t = {"act": 0.0, "dve": 0.0}  # greedy engine-time counters

    def qprep(g):
        """q-prep for heads 4g..4g+3. Squares+scales on DVE for group 0
        (warmup) and on GpSimd for group 1 (steady state)."""
        eng = nc.vector if g == 0 else nc.gpsimd
        x4 = io.tile([128, 4, NB, D], BF16, tag=f"x4_{g}", name=f"x4_{g}")
        nc.sync.dma_start(
            out=x4,
            in_=xq[4 * g : 4 * g + 4].rearrange("h (b p) d -> p h b d", p=128),
        )
        ss = st.tile([128, 4, NB], F32, tag=f"ss{g}", name=f"ss{g}")
        for hi in range(4):
            sq = st.tile([128, NB, D], BF16, tag=f"sq{g}", name=f"sq{g}")
            eng.tensor_mul(sq, x4[:, hi], x4[:, hi])
            nc.vector.tensor_reduce(ss[:, hi], sq, axis=AX.X, op=OP.add)

        # rs = sqrt_c2 / sqrt(ss + eps): fast inverse sqrt + 1 Newton (DVE)
        ssf = ss.rearrange("p h b -> p (h b)")
        half = st.tile([128, 32], F32, tag=f"half{g}", name=f"half{g}")
        nc.vector.tensor_scalar(
            out=half, in0=ssf, scalar1=0.5, scalar2=0.5 * EPS,
            op0=OP.mult, op1=OP.add,
        )
        rs = st.tile([128, 32, 1], F32, tag=f"rs{g}", name=f"rs{g}")
        rsf = rs.rearrange("p a one -> p (a one)")
        yi = rsf.bitcast(I32)
        nc.vector.tensor_scalar(
            out=yi, in0=ssf.bitcast(I32), scalar1=1, scalar2=None,
            op0=OP.logical_shift_right,
        )
        nc.vector.tensor_scalar(
            out=yi, in0=yi, scalar1=MAGIC, scalar2=-1,
            op0=OP.subtract, op1=OP.mult,
        )
        t1 = st.tile([128, 32], F32, tag=f"t1{g}", name=f"t1{g}")
        nc.vector.tensor_mul(t1, rsf, rsf)
        nc.vector.tensor_mul(t1, t1, half)
        nc.vector.tensor_scalar(
            out=t1, in0=t1, scalar1=1.5, scalar2=-sqrt_c2,
            op0=OP.subtract, op1=OP.mult,
        )
        nc.vector.tensor_mul(rsf, rsf, t1)

        # scale+cast to fp8 (one broadcast-multiply per head), transpose per
        # block (dispatch split across the SP and ACT queues for group 0),
        # then GpSimd-deinterleave into the DoubleRow block layout.
        q4 = qpool.tile([128, NB, 128], BF16, tag=f"q4_{g}", name=f"q4_{g}")
        qTi = qpool.tile([128, NB, 128], BF16, tag=f"qTi_{g}", name=f"qTi_{g}")
        qT = qpool.tile([128, N], BF16, tag=f"qT4_{g}", name=f"qT4_{g}")
        q4f = q4.bitcast(FP8)  # [128, NB, 256]
        qTif = qTi.bitcast(FP8)
        qTf = qT.bitcast(FP8).rearrange("p (two t) -> p two t", two=2)
        for hi in range(4):
            eng.tensor_mul(
                q4f[:, :, hi * 64 : (hi + 1) * 64],
                x4[:, hi],
                rs[:, hi * NB : (hi + 1) * NB, :].broadcast_to([128, NB, D]),
            )
        for b in range(NB):
            dq = nc.scalar if (g == 0 and b % 2 == 1) else nc.sync
            dq.dma_start(out=qTi[:, b, :], in_=q4[:, b, :], transpose=True)
            if b % 4 == 3:
                b0 = b - 3
                nc.gpsimd.tensor_copy(
                    qTf[:, :, b0 * 128 : (b0 + 4) * 128],
                    qTif[:, b0 : b0 + 4, :].rearrange(
                        "p b (t two) -> p two (b t)", two=2
                    ),
                )
        qT4[g] = qT

    def q_lhsT(g, hi, i):
        f = qT4[g].bitcast(FP8).rearrange("p (two t) -> p two t", two=2)
        return f[hi * 32 : (hi + 1) * 32, :, i * 128 : (i + 1) * 128]

    def q_rhs(g, hi, jc):
        f = qT4[g].bitcast(FP8).rearrange("p (two t) -> p two t", two=2)
        return f[hi * 32 : (hi + 1) * 32, :, jc * 512 : (jc + 1) * 512]

    wt_sb = singles.tile([D + 1, D], BF16)

    qprep(0)
    nc.sync.dma_start(out=wt_sb, in_=wt[:, :])

    def load_head(h):
        ya_t = io.tile([128, NB, D + 1], BF16, tag="ya", name=f"ya{h}")
        nc.sync.dma_start(
            out=ya_t, in_=ya[h].rearrange("(b p) d -> p b d", p=128)
        )
        yb_t = io.tile([128, NB, D], F32, tag="yb", name=f"yb{h}")
        nc.sync.dma_start(
            out=yb_t, in_=yb[h].rearrange("(b p) d -> p b d", p=128)
        )
        return ya_t, yb_t

    pend = load_head(0)

    for h in range(H):
        g, hi = h // 4, h % 4
        ya_t, yb_t = pend
        E = epool.tile([128, NW], BF16, tag="E", name=f"E{h}")
        Ei = E.bitcast(I16)
        po = ps_o.tile([128, 1024], F32, tag="o", name=f"o{h}")
        OT = wpool.tile([D + 1, N], BF16, tag="OT", name=f"OT{h}")
        okptr = [0]

        def emit_o(limit):
            while okptr[0] < 16:
                k = okptr[0]
                jc, i = k // NB, k % NB
                if jc * 4096 + (i + 1) * 512 > limit:
                    return
                nc.tensor.matmul(
                    po[0 : D + 1, jc * 512 : (jc + 1) * 512],
                    lhsT=ya_t[:, i, :],
                    rhs=E[:, jc * 4096 + i * 512 : jc * 4096 + (i + 1) * 512],
                    start=(i == 0), stop=(i == NB - 1), tile_position=(0, 0),
                )
                okptr[0] += 1

        def evac(jc):
            src = po[0 : D + 1, jc * 512 : (jc + 1) * 512]
            dst = OT[:, jc * 512 : (jc + 1) * 512]
            if eng_t["act"] + ACT_EVAC <= eng_t["dve"] + DVE_EVAC:
                nc.scalar.copy(out=dst, in_=src)
                eng_t["act"] += ACT_EVAC
            else:
                nc.vector.tensor_copy(dst, src)
                eng_t["dve"] += DVE_EVAC

        def emit_proj(jc):
            for bb in range(4):
                b = jc * 4 + bb
                nc.tensor.matmul(
                    po[:, jc * 512 + bb * 64 : jc * 512 + (bb + 1) * 64],
                    lhsT=OT[:, b * 128 : (b + 1) * 128],
                    rhs=wt_sb,
                    start=True, stop=True, tile_position=(0, 0),
                )

        for c in range(NB):
            jc, ip = c // 4, (c % 4) * 2
            ps = ps_s.tile([128, CHUNK], F32, tag="psS", name="psS")
            for k in range(2):
                nc.tensor.matmul(
                    ps[:, k * 512 : (k + 1) * 512],
                    lhsT=q_lhsT(g, hi, ip + k),
                    rhs=q_rhs(g, hi, jc),
                    start=True, stop=True,
                    perf_mode=PM.DoubleRow,
                    tile_position=(hi * 32, 0),
                )
            if eng_t["dve"] + DVE_CHUNK < eng_t["act"] + ACT_CHUNK:
                nc.vector.tensor_scalar(
                    out=Ei[:, c * CHUNK : (c + 1) * CHUNK],
                    in0=ps, scalar1=A_SCH, scalar2=B_SCH,
                    op0=OP.mult, op1=OP.add,
                )
                eng_t["dve"] += DVE_CHUNK
            else:
                nc.scalar.activation(
                    out=E[:, c * CHUNK : (c + 1) * CHUNK], in_=ps, func=ACT.Exp
                )
                eng_t["act"] += ACT_CHUNK
            emit_o(c * CHUNK)
            if c == 0:
                if h + 1 < H:
                    pend = load_head(h + 1)
                if h == 2:
                    qprep(1)
            elif c == 5:
                evac(0)
            elif c == 6:
                emit_proj(0)

        # ---- head tail: O flush, denominators, second half, epilogue ----
        emit_o(NW)
        evac(1)
        nc.sync.dma_start(out=rscr[h], in_=OT[D : D + 1, :])
        rT = st.tile([128, NB], BF16, tag="rT", name=f"rT{h}")
        nc.sync.dma_start(out=rT, in_=rscr[h].rearrange("(b p) -> p b", p=128))
        emit_proj(1)

        rinv = st.tile([128, NB, 1], F32, tag="rinv", name=f"rinv{h}")
        nc.vector.reciprocal(rinv.rearrange("p a one -> p (a one)"), rT)
        eng_t["dve"] += DVE_RECIP

        pj = (
            po.rearrange("p (jc x) -> p jc x", jc=2)[:, :, 0:256]
            .rearrange("p jc (bb d) -> p jc bb d", bb=4)
        )
        rb = (
            rinv.rearrange("p (jc bb) one -> p jc bb one", jc=2)
            .broadcast_to([128, 2, 4, D])
        )
        o2 = wpool.tile([128, NB, D], F32, tag="o2", name=f"o2{h}")
        nc.vector.tensor_mul(
            o2.rearrange("p (jc bb) d -> p jc bb d", jc=2), pj, rb
        )
        eng_t["dve"] += DVE_O2
        fin = wpool.tile([128, NB, D], F32, tag="fin", name=f"fin{h}")
        nc.gpsimd.tensor_add(fin, o2, yb_t)
        nc.sync.dma_start(
            out=out[h].rearrange("(b p) d -> p b d", p=128), in_=fin
        )


def build_program(sqrt_c2: float) -> bass.Bass:
    from contextlib import ExitStack

    nc = bacc.Bacc(get_trn_type() or "TRN2", target_bir_lowering=False)
    with tile.TileContext(nc) as tc:
        with ExitStack() as ctx:
            _emit(ctx, tc, sqrt_c2)
    nc.compile()
    return nc


def make_inputs(x, y, proj_w, proj_b, attn_gamma, sum_gamma0, sum_gamma1):
    x = np.asarray(x, dtype=np.float32)
    y = np.asarray(y, dtype=np.float32)
    proj_w = np.asarray(proj_w, dtype=np.float32)
    proj_b = np.asarray(proj_b, dtype=np.float32)
    g0 = math.exp(float(np.asarray(sum_gamma0)))
    g1 = math.exp(float(np.asarray(sum_gamma1)))
    w0 = g0 / (g0 + g1)
    w1 = g1 / (g0 + g1)
    c2 = 1.0 / (SCALE * float(np.asarray(attn_gamma)))

    xq = x.astype(ml_dtypes.bfloat16)
    yac = np.concatenate(
        [y, np.ones(y.shape[:-1] + (1,), np.float32)], axis=-1
    ).astype(ml_dtypes.bfloat16)
    ybv = (w0 * y).astype(np.float32)
    wtv = np.concatenate([proj_w.T * w1, w1 * proj_b[None, :]], axis=0).astype(
        ml_dtypes.bfloat16
    )
    in_maps = [
        {"xq": xq[c], "ya": yac[c], "yb": ybv[c], "wt": wtv}
        for c in range(NCORES)
    ]
    return in_maps, math.sqrt(c2)


def kernel(x, y, proj_w, proj_b, attn_gamma, sum_gamma0, sum_gamma1):
    global LAST_RESULTS
    in_maps, sqrt_c2 = make_inputs(
        x, y, proj_w, proj_b, attn_gamma, sum_gamma0, sum_gamma1
    )
    nc = build_program(sqrt_c2)
    res = run_bass_kernel_spmd(nc, in_maps, list(range(NCORES)))
    LAST_RESULTS = res
    return np.stack([res.results[c]["out"] for c in range(NCORES)], axis=0)
